# revision 2
# baseline (speedup 1.0000x reference)
"""GCN layer on 8 TRN2 NeuronCores.

    out = relu( D^-1/2 (A+I) D^-1/2 @ (x @ W) + b ),  A = adj (dense), d = rowsum(A+I)

Sharding: 1-D row partition of nodes across 8 cores (core c owns rows
r_c = [c*1024, (c+1)*1024)).  Each core receives its adjacency rows
pre-transposed (adjT = adj[r_c,:].T, the natural stationary-operand layout
for the tensor engine) and its x rows pre-transposed (xT = x[r_c,:].T).

Per-core device program (SPMD, identical on all cores):
  1. pre = x[r_c] @ W                      (TensorE, bf16)
  2. AllGather(pre)   -> P [8192, 512]     (unscaled; overlaps step 3)
  3. d_raw[m] = sum_k adj[r_c,:][m,k]      (TensorE: ones.T @ adjT tiles)
  4. AllGather(d_raw) -> d_all [8192]
  5. d_is = 1/sqrt(d_all + 1)              (ACT sqrt + DVE reciprocal)
  6. big = adjT.T @ (d_is * P)             (TensorE, bf16, k-streamed)
  7. out = relu(d_is_loc*(big + d_is_loc*pre) + b)   (DVE + ACT)

All matmul compute in bf16 with fp32 PSUM accumulation; d/scales in fp32.
"""

import sys

if "/opt/trn_rl_repo" not in sys.path:
    sys.path.insert(0, "/opt/trn_rl_repo")

import numpy as np
import ml_dtypes

N = 8192
F_IN = 1024
F_OUT = 512
NCORES = 8
ROWS = N // NCORES          # 1024 rows per core
KT = N // 128               # 64 k-tiles over nodes
FT = F_IN // 128            # 8 f-tiles over input features
MT = ROWS // 128            # 8 m-tiles over local rows

_COMPILED = None


def _build():
    import concourse.bass as bass
    import concourse.mybir as mybir
    import concourse.tile as tile
    from concourse import bacc

    dt = mybir.dt
    Alu = mybir.AluOpType
    Act = mybir.ActivationFunctionType

    nc = bacc.Bacc("TRN2", target_bir_lowering=False, debug=False,
                   num_devices=NCORES)

    adjT_ext = nc.dram_tensor("adjT", [N, ROWS], dt.bfloat16, kind="ExternalInput")
    xT_ext = nc.dram_tensor("xT", [F_IN, ROWS], dt.bfloat16, kind="ExternalInput")
    w_ext = nc.dram_tensor("Wm", [F_IN, F_OUT], dt.bfloat16, kind="ExternalInput")
    b_ext = nc.dram_tensor("bias", [F_OUT], dt.float32, kind="ExternalInput")
    out_ext = nc.dram_tensor("out", [ROWS, F_OUT], dt.float32, kind="ExternalOutput")

    rg = [list(range(NCORES))]

    with tile.TileContext(nc) as tc:
        with (
            tc.tile_pool(name="wpool", bufs=1) as wpool,
            tc.tile_pool(name="xpool", bufs=1) as xpool,
            tc.tile_pool(name="prepool", bufs=1) as prepool,
            tc.tile_pool(name="adjpool", bufs=4) as adjpool,
            tc.tile_pool(name="ppool", bufs=4) as ppool,
            tc.tile_pool(name="spool", bufs=4) as spool,
            tc.tile_pool(name="epool", bufs=2) as epool,
            tc.tile_pool(name="misc", bufs=1) as misc,
            tc.tile_pool(name="dram", bufs=1, space="DRAM") as dram,
        ):
            # ---- phase A: pre = x @ W ------------------------------------
            w_sb = []
            xt_sb = []
            for f in range(FT):
                wt = wpool.tile([128, F_OUT], dt.bfloat16, name=f"w{f}")
                nc.sync.dma_start(wt[:], w_ext[f * 128:(f + 1) * 128, :])
                w_sb.append(wt)
                xt = xpool.tile([128, ROWS], dt.bfloat16, name=f"xt{f}")
                nc.sync.dma_start(xt[:], xT_ext[f * 128:(f + 1) * 128, :])
                xt_sb.append(xt)

            pre_bounce = dram.tile([ROWS, F_OUT], dt.bfloat16)
            pre_sb = []
            with (
                tc.tile_pool(name="pre_psum", bufs=2, space="PSUM") as pre_psum,
                tc.tile_pool(name="d_psum", bufs=1, space="PSUM") as d_psum,
            ):
                for m in range(MT):
                    ps = pre_psum.tile([128, F_OUT], dt.float32, name="ps_pre",
                                       tag="ps_pre")
                    for f in range(FT):
                        nc.tensor.matmul(
                            ps[:], xt_sb[f][:, m * 128:(m + 1) * 128], w_sb[f][:],
                            start=(f == 0), stop=(f == FT - 1),
                        )
                    pre = prepool.tile([128, F_OUT], dt.bfloat16, name=f"pre{m}")
                    nc.vector.tensor_copy(pre[:], ps[:])
                    pre_sb.append(pre)
                    nc.sync.dma_start(pre_bounce[m * 128:(m + 1) * 128, :], pre[:])

                # ---- AllGather pre (unscaled) ----------------------------
                pre_all = dram.tile([N, F_OUT], dt.bfloat16, addr_space="Shared")
                nc.gpsimd.collective_compute(
                    "AllGather", Alu.bypass, replica_groups=rg,
                    ins=[pre_bounce[:].opt()], outs=[pre_all[:].opt()],
                )

                # ---- phase A: d_raw = rowsum(adj[r_c,:]) -----------------
                ones = misc.tile([128, 1], dt.bfloat16)
                nc.vector.memset(ones[:], 1.0)
                dps0 = d_psum.tile([1, F_OUT], dt.float32, name="dps0")
                dps1 = d_psum.tile([1, F_OUT], dt.float32, name="dps1")
                for k in range(KT):
                    at = adjpool.tile([128, ROWS], dt.bfloat16, name="adjt",
                                      tag="adjt")
                    nc.sync.dma_start(at[:], adjT_ext[k * 128:(k + 1) * 128, :])
                    nc.tensor.matmul(dps0[:], ones[:], at[:, 0:512],
                                     start=(k == 0), stop=(k == KT - 1))
                    nc.tensor.matmul(dps1[:], ones[:], at[:, 512:1024],
                                     start=(k == 0), stop=(k == KT - 1))
                d_sb = misc.tile([1, ROWS], dt.float32)
                nc.vector.tensor_copy(d_sb[:, 0:512], dps0[:])
                nc.vector.tensor_copy(d_sb[:, 512:1024], dps1[:])
                d_bounce = dram.tile([ROWS], dt.float32)
                nc.sync.dma_start(d_bounce[None, :], d_sb[:])

            # ---- AllGather d ---------------------------------------------
            d_all = dram.tile([N], dt.float32, addr_space="Shared")
            nc.gpsimd.collective_compute(
                "AllGather", Alu.bypass, replica_groups=rg,
                ins=[d_bounce[:].opt()], outs=[d_all[:].opt()],
            )

            # ---- d_is = 1/sqrt(d+1), striped [p, t] ----------------------
            dt_all = misc.tile([128, KT], dt.float32)
            nc.sync.dma_start(dt_all[:], d_all[:].rearrange("(t p) -> p t", p=128))
            sq_all = misc.tile([128, KT], dt.float32)
            nc.scalar.activation(sq_all[:], dt_all[:], Act.Sqrt, bias=1.0)
            dis_all = misc.tile([128, KT], dt.float32)
            nc.vector.reciprocal(dis_all[:], sq_all[:])

            dt_loc = misc.tile([128, MT], dt.float32)
            nc.sync.dma_start(dt_loc[:], d_bounce[:].rearrange("(t p) -> p t", p=128))
            sq_loc = misc.tile([128, MT], dt.float32)
            nc.scalar.activation(sq_loc[:], dt_loc[:], Act.Sqrt, bias=1.0)
            dis_loc = misc.tile([128, MT], dt.float32)
            nc.vector.reciprocal(dis_loc[:], sq_loc[:])

            # bias broadcast to all partitions
            b_sb = misc.tile([128, F_OUT], dt.float32)
            nc.sync.dma_start(b_sb[:], b_ext[None, :].to_broadcast((128, F_OUT)))

            # ---- big matmul: adjT.T @ (d_is * P) -------------------------
            with tc.tile_pool(name="big_psum", bufs=1, space="PSUM") as big_psum:
                big = [big_psum.tile([128, F_OUT], dt.float32, name=f"big{m}")
                       for m in range(MT)]
                for k in range(KT):
                    at = adjpool.tile([128, ROWS], dt.bfloat16, name="adjt2",
                                      tag="adjt")
                    nc.sync.dma_start(at[:], adjT_ext[k * 128:(k + 1) * 128, :])
                    pk = ppool.tile([128, F_OUT], dt.bfloat16, name="pk", tag="pk")
                    nc.sync.dma_start(pk[:], pre_all[k * 128:(k + 1) * 128, :])
                    pks = spool.tile([128, F_OUT], dt.bfloat16, name="pks",
                                     tag="pks")
                    nc.vector.tensor_scalar_mul(pks[:], pk[:], dis_all[:, k:k + 1])
                    for m in range(MT):
                        nc.tensor.matmul(
                            big[m][:], at[:, m * 128:(m + 1) * 128], pks[:],
                            start=(k == 0), stop=(k == KT - 1),
                        )

                # ---- epilogue --------------------------------------------
                for m in range(MT):
                    t1 = epool.tile([128, F_OUT], dt.float32, name="t1", tag="t1")
                    # t1 = pre*dis_loc + big   (self-loop term + propagation)
                    nc.vector.scalar_tensor_tensor(
                        t1[:], pre_sb[m][:], dis_loc[:, m:m + 1], big[m][:],
                        op0=Alu.mult, op1=Alu.add,
                    )
                    t2 = epool.tile([128, F_OUT], dt.float32, name="t2", tag="t2")
                    # t2 = t1*dis_loc + b
                    nc.vector.scalar_tensor_tensor(
                        t2[:], t1[:], dis_loc[:, m:m + 1], b_sb[:],
                        op0=Alu.mult, op1=Alu.add,
                    )
                    ot = epool.tile([128, F_OUT], dt.float32, name="ot", tag="ot")
                    nc.scalar.activation(ot[:], t2[:], Act.Relu)
                    nc.sync.dma_start(out_ext[m * 128:(m + 1) * 128, :], ot[:])

    nc.compile()
    return nc


def _get_compiled():
    global _COMPILED
    if _COMPILED is None:
        _COMPILED = _build()
    return _COMPILED


def kernel(x, adj, W, b):
    from concourse.bass_utils import run_bass_kernel_spmd

    x = np.asarray(x)
    adj = np.asarray(adj)
    W = np.asarray(W)
    b = np.asarray(b)

    bf16 = ml_dtypes.bfloat16
    W_bf = np.ascontiguousarray(W.astype(bf16))
    b_f32 = np.ascontiguousarray(b.astype(np.float32))

    in_maps = []
    for c in range(NCORES):
        rows = slice(c * ROWS, (c + 1) * ROWS)
        in_maps.append({
            "adjT": adj[rows, :].T.astype(bf16),   # [N, ROWS] contiguous bf16
            "xT": x[rows, :].T.astype(bf16),       # [F_IN, ROWS]
            "Wm": W_bf,
            "bias": b_f32,
        })

    nc = _get_compiled()
    res = run_bass_kernel_spmd(nc, in_maps, list(range(NCORES)))
    return np.concatenate([res.results[c]["out"] for c in range(NCORES)], axis=0)


if __name__ == "__main__":
    rng = np.random.default_rng(0)
    x = rng.standard_normal((N, F_IN), dtype=np.float32)
    adj = rng.random((N, N), dtype=np.float32)
    W = rng.standard_normal((F_IN, F_OUT), dtype=np.float32) * 0.04
    b = np.zeros((F_OUT,), dtype=np.float32)
    out = kernel(x=x, adj=adj, W=W, b=b)
    print("out", out.shape, out.dtype, float(np.abs(out).max()))


# revision 3
# speedup vs baseline: 1.1220x; 1.1220x over previous
"""GCN layer on 8 TRN2 NeuronCores.

    out = relu( D^-1/2 (A+I) D^-1/2 @ (x @ W) + b ),  A = adj (dense), d = rowsum(A+I)

Sharding: 1-D row partition of nodes across 8 cores (core c owns rows
r_c = [c*1024, (c+1)*1024)).  Each core receives its adjacency rows
pre-transposed (adjT = adj[r_c,:].T, the natural stationary-operand layout
for the tensor engine) and its x rows pre-transposed (xT = x[r_c,:].T).

Per-core device program (SPMD, identical on all cores):
  1. pre = x[r_c] @ W                      (TensorE, bf16)
  2. AllGather(pre)   -> P [8192, 512]     (unscaled; overlaps step 3)
  3. d_raw[m] = sum_k adj[r_c,:][m,k]      (TensorE: ones.T @ adjT tiles,
                                            piggybacked on the single adjT
                                            HBM pass; adjT stays resident
                                            in SBUF: 64 tiles x 2KB/parti)
  4. AllGather(d_raw) -> d_all [8192]
  5. d_is = 1/sqrt(d_all + 1)              (PE transpose + ACT sqrt + DVE recip)
  6. big = adjT.T @ (d_is * P)             (TensorE, bf16; adjT from SBUF,
                                            P streamed ~0.5us/tile << PE
                                            1.7us/tile -> PE-bound)
  7. out = relu(d_is_loc*(big + d_is_loc*pre) + b)   (DVE + ACT)

All matmul compute in bf16 with fp32 PSUM accumulation; d/scales in fp32.
"""

import sys

if "/opt/trn_rl_repo" not in sys.path:
    sys.path.insert(0, "/opt/trn_rl_repo")

import numpy as np
import ml_dtypes

N = 8192
F_IN = 1024
F_OUT = 512
NCORES = 8
ROWS = N // NCORES          # 1024 rows per core
KT = N // 128               # 64 k-tiles over nodes
FT = F_IN // 128            # 8 f-tiles over input features
MT = ROWS // 128            # 8 m-tiles over local rows

_COMPILED = None


def _build():
    import concourse.bass as bass
    import concourse.mybir as mybir
    import concourse.tile as tile
    from concourse import bacc
    from concourse.masks import make_identity

    dt = mybir.dt
    Alu = mybir.AluOpType
    Act = mybir.ActivationFunctionType

    nc = bacc.Bacc("TRN2", target_bir_lowering=False, debug=False,
                   num_devices=NCORES)

    adjT_ext = nc.dram_tensor("adjT", [N, ROWS], dt.bfloat16, kind="ExternalInput")
    xT_ext = nc.dram_tensor("xT", [F_IN, ROWS], dt.bfloat16, kind="ExternalInput")
    w_ext = nc.dram_tensor("Wm", [F_IN, F_OUT], dt.bfloat16, kind="ExternalInput")
    b_ext = nc.dram_tensor("bias", [F_OUT], dt.float32, kind="ExternalInput")
    out_ext = nc.dram_tensor("out", [ROWS, F_OUT], dt.float32, kind="ExternalOutput")

    rg = [list(range(NCORES))]

    with tile.TileContext(nc) as tc:
        with (
            tc.tile_pool(name="adjpool", bufs=1) as adjpool,
            tc.tile_pool(name="prepool", bufs=1) as prepool,
            tc.tile_pool(name="misc", bufs=1) as misc,
            tc.tile_pool(name="dram", bufs=1, space="DRAM") as dram,
        ):
            pre_bounce = dram.tile([ROWS, F_OUT], dt.bfloat16)
            pre_all = dram.tile([N, F_OUT], dt.bfloat16, addr_space="Shared")
            d_bounce = dram.tile([ROWS], dt.float32)
            d_all = dram.tile([N], dt.float32, addr_space="Shared")

            pre_sb = []
            adj_sb = []
            with (
                tc.tile_pool(name="xwpool", bufs=1) as xwpool,
                tc.tile_pool(name="pre_psum", bufs=2, space="PSUM") as pre_psum,
                tc.tile_pool(name="d_psum", bufs=1, space="PSUM") as d_psum,
            ):
                # ---- pre = x @ W ----------------------------------------
                w_sb = []
                xt_sb = []
                for f in range(FT):
                    wt = xwpool.tile([128, F_OUT], dt.bfloat16, name=f"w{f}")
                    nc.sync.dma_start(wt[:], w_ext[f * 128:(f + 1) * 128, :])
                    w_sb.append(wt)
                    xt = xwpool.tile([128, ROWS], dt.bfloat16, name=f"xt{f}")
                    nc.sync.dma_start(xt[:], xT_ext[f * 128:(f + 1) * 128, :])
                    xt_sb.append(xt)

                for m in range(MT):
                    ps = pre_psum.tile([128, F_OUT], dt.float32, name="ps_pre",
                                       tag="ps_pre")
                    for f in range(FT):
                        nc.tensor.matmul(
                            ps[:], xt_sb[f][:, m * 128:(m + 1) * 128], w_sb[f][:],
                            start=(f == 0), stop=(f == FT - 1),
                        )
                    pre = prepool.tile([128, F_OUT], dt.bfloat16, name=f"pre{m}")
                    nc.vector.tensor_copy(pre[:], ps[:])
                    pre_sb.append(pre)
                    nc.sync.dma_start(pre_bounce[m * 128:(m + 1) * 128, :], pre[:])

                # ---- AllGather pre (unscaled) ---------------------------
                nc.gpsimd.collective_compute(
                    "AllGather", Alu.bypass, replica_groups=rg,
                    ins=[pre_bounce[:].opt()], outs=[pre_all[:].opt()],
                )

                # ---- single adjT HBM pass: keep resident + row sums -----
                ones = misc.tile([128, 1], dt.bfloat16)
                nc.vector.memset(ones[:], 1.0)
                dps0 = d_psum.tile([1, F_OUT], dt.float32, name="dps0")
                dps1 = d_psum.tile([1, F_OUT], dt.float32, name="dps1")
                for k in range(KT):
                    at = adjpool.tile([128, ROWS], dt.bfloat16, name=f"adjt{k}")
                    nc.sync.dma_start(at[:], adjT_ext[k * 128:(k + 1) * 128, :])
                    adj_sb.append(at)
                    nc.tensor.matmul(dps0[:], ones[:], at[:, 0:512],
                                     start=(k == 0), stop=(k == KT - 1))
                    nc.tensor.matmul(dps1[:], ones[:], at[:, 512:1024],
                                     start=(k == 0), stop=(k == KT - 1))
                d_sb = misc.tile([1, ROWS], dt.float32)
                nc.vector.tensor_copy(d_sb[:, 0:512], dps0[:])
                nc.vector.tensor_copy(d_sb[:, 512:1024], dps1[:])
                nc.sync.dma_start(d_bounce[None, :], d_sb[:])

                # ---- AllGather d ----------------------------------------
                nc.gpsimd.collective_compute(
                    "AllGather", Alu.bypass, replica_groups=rg,
                    ins=[d_bounce[:].opt()], outs=[d_all[:].opt()],
                )

                # local d_is while AG-d is in flight (depends on d_bounce only)
                dt_loc = misc.tile([128, MT], dt.float32)
                nc.scalar.dma_start(
                    dt_loc[:], d_bounce[:].rearrange("(t p) -> p t", p=128))
                sq_loc = misc.tile([128, MT], dt.float32)
                nc.scalar.activation(sq_loc[:], dt_loc[:], Act.Sqrt, bias=1.0)
                dis_loc = misc.tile([128, MT], dt.float32)
                nc.vector.reciprocal(dis_loc[:], sq_loc[:])

                # bias broadcast to all partitions
                b_sb = misc.tile([128, F_OUT], dt.float32)
                nc.scalar.dma_start(b_sb[:], b_ext[None, :].to_broadcast((128, F_OUT)))

            # ---- d_is_all = 1/sqrt(d_all + 1), striped [p, t] ------------
            dis_all = misc.tile([128, KT], dt.float32)
            with (
                tc.tile_pool(name="tpool", bufs=1) as tpool,
                tc.tile_pool(name="t_psum", bufs=1, space="PSUM") as t_psum,
            ):
                ident = tpool.tile([KT, KT], dt.float32)
                make_identity(nc, ident[:])
                dflat = tpool.tile([KT, 128], dt.float32)
                # d_all[k] with k = t*128 + p is a contiguous [KT, 128] matrix
                nc.scalar.dma_start(dflat[:], d_all[:].rearrange("(t p) -> t p", p=128))
                dtr = t_psum.tile([128, KT], dt.float32)
                nc.tensor.transpose(dtr[:], dflat[:], ident[:])
                sq_all = tpool.tile([128, KT], dt.float32)
                nc.scalar.activation(sq_all[:], dtr[:], Act.Sqrt, bias=1.0)
                nc.vector.reciprocal(dis_all[:], sq_all[:])

            # ---- big matmul: adjT.T @ (d_is * P), adjT from SBUF ---------
            with (
                tc.tile_pool(name="ppool", bufs=16) as ppool,
                tc.tile_pool(name="spool", bufs=8) as spool,
                tc.tile_pool(name="epool", bufs=2) as epool,
                tc.tile_pool(name="big_psum", bufs=1, space="PSUM") as big_psum,
            ):
                big = [big_psum.tile([128, F_OUT], dt.float32, name=f"big{m}")
                       for m in range(MT)]
                for k in range(KT):
                    pk = ppool.tile([128, F_OUT], dt.bfloat16, name="pk", tag="pk")
                    nc.sync.dma_start(pk[:], pre_all[k * 128:(k + 1) * 128, :])
                    pks = spool.tile([128, F_OUT], dt.bfloat16, name="pks",
                                     tag="pks")
                    nc.vector.tensor_scalar_mul(pks[:], pk[:], dis_all[:, k:k + 1])
                    for m in range(MT):
                        nc.tensor.matmul(
                            big[m][:], adj_sb[k][:, m * 128:(m + 1) * 128], pks[:],
                            start=(k == 0), stop=(k == KT - 1),
                        )

                # ---- epilogue --------------------------------------------
                for m in range(MT):
                    t1 = epool.tile([128, F_OUT], dt.float32, name="t1", tag="t1")
                    # t1 = pre*dis_loc + big   (self-loop term + propagation)
                    nc.vector.scalar_tensor_tensor(
                        t1[:], pre_sb[m][:], dis_loc[:, m:m + 1], big[m][:],
                        op0=Alu.mult, op1=Alu.add,
                    )
                    t2 = epool.tile([128, F_OUT], dt.float32, name="t2", tag="t2")
                    # t2 = t1*dis_loc + b
                    nc.vector.scalar_tensor_tensor(
                        t2[:], t1[:], dis_loc[:, m:m + 1], b_sb[:],
                        op0=Alu.mult, op1=Alu.add,
                    )
                    ot = epool.tile([128, F_OUT], dt.float32, name="ot", tag="ot")
                    nc.scalar.activation(ot[:], t2[:], Act.Relu)
                    nc.sync.dma_start(out_ext[m * 128:(m + 1) * 128, :], ot[:])

    nc.compile()
    return nc


def _get_compiled():
    global _COMPILED
    if _COMPILED is None:
        _COMPILED = _build()
    return _COMPILED


def kernel(x, adj, W, b):
    from concourse.bass_utils import run_bass_kernel_spmd

    x = np.asarray(x)
    adj = np.asarray(adj)
    W = np.asarray(W)
    b = np.asarray(b)

    bf16 = ml_dtypes.bfloat16
    W_bf = np.ascontiguousarray(W.astype(bf16))
    b_f32 = np.ascontiguousarray(b.astype(np.float32))

    in_maps = []
    for c in range(NCORES):
        rows = slice(c * ROWS, (c + 1) * ROWS)
        in_maps.append({
            "adjT": adj[rows, :].T.astype(bf16),   # [N, ROWS] contiguous bf16
            "xT": x[rows, :].T.astype(bf16),       # [F_IN, ROWS]
            "Wm": W_bf,
            "bias": b_f32,
        })

    nc = _get_compiled()
    res = run_bass_kernel_spmd(nc, in_maps, list(range(NCORES)))
    return np.concatenate([res.results[c]["out"] for c in range(NCORES)], axis=0)


if __name__ == "__main__":
    rng = np.random.default_rng(0)
    x = rng.standard_normal((N, F_IN), dtype=np.float32)
    adj = rng.random((N, N), dtype=np.float32)
    W = rng.standard_normal((F_IN, F_OUT), dtype=np.float32) * 0.04
    b = np.zeros((F_OUT,), dtype=np.float32)
    out = kernel(x=x, adj=adj, W=W, b=b)
    print("out", out.shape, out.dtype, float(np.abs(out).max()))
